# revision 1
# baseline (speedup 1.0000x reference)
"""Trainium2 Bass kernel for nn_Neighbor_Mean (gnn message passing).

Math: out[b,s,:] = mean_n( mask[b,s,n] * (T_b[idx[b,s,n]] @ Wn^T) )
 with T_b[v] = pos_table[v] + (h[b][v-1] if v>=1 else 0)   (v in [0, 2049))
Since the mask multiplies matmul outputs and matmul is linear:
 out[b,s,:] = ( (1/N) * sum_n T'_b[idx_eff[b,s,n]] ) @ Wn^T
 where T' has an extra zero row at SINK=2049 and idx_eff = mask ? idx : SINK.

Sharding: data-parallel over batch, one NeuronCore per batch row (B == 8).

Per-core plan:
 - build T' in SBUF as bf16, packed [128 part, 17*128] (row v at partition
   v%128, free chunk v//128) -- the SBUF-source layout of dma_gather
   (tokens_per_rank=128, free_dim_per_rank=256B).
 - fold mask into indices on DVE (select against SINK), emit int16 in the
   16-partition wrapped layout dma_gather wants, replicate to 128 partitions.
 - SBUF->SBUF transposed dma_gather, 512 idx/call (ucode ring ceiling),
   4 SWDGE queues. Gathered tile g[h=128 part, j free], stream
   j = (n, s%16) per call, call c covers s in [16c, 16c+16).
 - PE: per 128-s chunk, PSUM-accumulate 32 matmuls over n:
   psum[s,k] += g_slice[h, s]^T @ (Wn^T * 1/N) (bf16 x bf16 -> f32).
 - copy PSUM->SBUF, DMA out rows (f32).
"""
import sys

sys.path.insert(0, '/opt/trn_rl_repo')

import numpy as np

import concourse.bacc as bacc
import concourse.bass as bass
import concourse.mybir as mybir
import concourse.tile as tile
from concourse.bass_utils import run_bass_kernel_spmd
from concourse.masks import make_identity

B, N, H = 8, 32, 128
NI = 512             # idxs per dma_gather call (ucode ring ceiling)
SBLK = 512           # s rows per pipeline block
TPR = 128            # sbuf gather tokens per rank
F32 = mybir.dt.float32
I32 = mybir.dt.int32
I16 = mybir.dt.int16
BF16 = mybir.dt.bfloat16


def build_program(S: int = 2048):
    VPOS = S + 1                      # pos_table rows; SINK index == VPOS
    NRANKS = (VPOS + 1 + 127) // 128  # table chunks incl. sink row, padded
    VPAD = NRANKS * 128
    nblk = S // SBLK if S >= SBLK else 1
    sblk = min(SBLK, S)
    calls = sblk * N // NI            # gather calls per block
    chunks = sblk // 128              # 128-s output chunks per block

    nc = bacc.Bacc("TRN2", debug=False, num_swdge_queues=4)
    h_d = nc.dram_tensor("h", [S, H], F32, kind="ExternalInput")
    idx_d = nc.dram_tensor("idx", [S, N], I32, kind="ExternalInput")
    msk_d = nc.dram_tensor("msk", [S, N], I32, kind="ExternalInput")
    pos_d = nc.dram_tensor("pos", [VPOS, H], F32, kind="ExternalInput")
    wn_d = nc.dram_tensor("wn", [H, H], F32, kind="ExternalInput")
    out_d = nc.dram_tensor("out", [S, H], F32, kind="ExternalOutput")

    with tile.TileContext(nc) as tc:
        with (
            tc.tile_pool(name="const", bufs=1) as constp,
            tc.tile_pool(name="stage", bufs=3) as stagep,
            tc.tile_pool(name="idxp", bufs=2) as idxp,
            tc.tile_pool(name="gbig", bufs=2) as gbigp,
            tc.tile_pool(name="outp", bufs=4) as outp,
            tc.tile_pool(name="psum", bufs=4, space="PSUM") as psump,
        ):
            # ---- Wn^T * (1/N) in bf16 --------------------------------
            wn_sb = constp.tile([H, H], F32)
            nc.sync.dma_start(wn_sb[:], wn_d[:])
            ident = constp.tile([128, 128], F32)
            make_identity(nc, ident[:])
            wnt_ps = psump.tile([128, H], F32)
            nc.tensor.transpose(out=wnt_ps[:], in_=wn_sb[:], identity=ident[:])
            wnt = constp.tile([H, H], BF16)
            nc.vector.tensor_scalar_mul(wnt[:], wnt_ps[:], 1.0 / N)

            # ---- fused table T' (bf16, gather-packed layout) ---------
            # tbl[p, q*H:(q+1)*H] = T'[q*128 + p, :]
            tbl = constp.tile([128, NRANKS * H], BF16)
            for q in range(NRANKS):
                v0 = q * 128
                n_pos = min(128, VPOS - v0)       # valid pos rows this chunk
                if n_pos <= 0:
                    nc.gpsimd.memset(tbl[:, q * H:(q + 1) * H], 0.0)
                    continue
                pstage = stagep.tile([128, H], F32, tag="pstage")
                hstage = stagep.tile([128, H], F32, tag="hstage")
                if n_pos < 128:
                    nc.gpsimd.memset(tbl[:, q * H:(q + 1) * H], 0.0)
                nc.sync.dma_start(pstage[:n_pos, :], pos_d[v0:v0 + n_pos, :])
                # h rows v0-1 .. v0+n_pos-2 ; row p needs h[v0+p-1]
                if q == 0:
                    nc.gpsimd.memset(hstage[0:1, :], 0.0)
                    nc.sync.dma_start(hstage[1:n_pos, :], h_d[0:n_pos - 1, :])
                else:
                    nc.sync.dma_start(hstage[:n_pos, :], h_d[v0 - 1:v0 + n_pos - 1, :])
                nc.vector.tensor_add(
                    tbl[:n_pos, q * H:(q + 1) * H], pstage[:n_pos, :], hstage[:n_pos, :]
                )

            # ---- wrapped masked indices (whole batch, prologue) ------
            # IMPORTANT: all 2-read DVE ops (copy_predicated) must finish
            # before any dma_gather runs -- the gather ucode streams its
            # indices through the POOL/DVE *shared* SBUF read port, and a
            # concurrent 2-port DVE op corrupts the stream. Hoisting the
            # whole index prep into the prologue makes every gather
            # transitively depend on it.
            #
            # gather call c = 8u + n_hi covers s in [128u, 128u+128) and
            # n in [4*n_hi, 4*n_hi+4); position in call i = 128*n_lo + s_lo,
            # so gbig column = 512*(n//4) + 128*(n%4) + s_lo per block.
            # Wrapped idx buffer [16, (u, n_hi, n_lo, s_hi)]:
            # idxw[p, 256u + 32*n_hi + 8*n_lo + s_hi]
            #   = idx_eff[128u + 16*s_hi + p, 4*n_hi + n_lo]
            acols = S * N // 16  # wrapped cols, whole batch
            c_sink = constp.tile([16, acols], I32)
            nc.gpsimd.memset(c_sink[:], VPOS)
            idxw32 = idxp.tile([16, acols], I32, tag="idxw32")
            mskw32 = idxp.tile([16, acols], I32, tag="mskw32")
            for u in range(S // 128):
                su = u * 128
                src_i = idx_d[su:su + 128, :].rearrange(
                    "(shi p) (nhi nlo) -> p nhi nlo shi", p=16, nlo=4)
                src_m = msk_d[su:su + 128, :].rearrange(
                    "(shi p) (nhi nlo) -> p nhi nlo shi", p=16, nlo=4)
                dst_i = idxw32[:, u * 256:(u + 1) * 256].rearrange(
                    "p (nhi nlo shi) -> p nhi nlo shi", nlo=4, shi=8)
                dst_m = mskw32[:, u * 256:(u + 1) * 256].rearrange(
                    "p (nhi nlo shi) -> p nhi nlo shi", nlo=4, shi=8)
                eng = nc.sync if u % 2 == 0 else nc.scalar
                eng.dma_start(dst_i, src_i)
                eng.dma_start(dst_m, src_m)
            idxe32 = idxp.tile([16, acols], I32, tag="idxe32")
            nc.vector.tensor_copy(idxe32[:], c_sink[:])
            nc.vector.copy_predicated(idxe32[:], mskw32[:], idxw32[:])
            # int32 -> int16 (values < 2^15: take low halves)
            idxbuf = idxp.tile([128, acols], I16, tag="idxbuf")
            lo = idxe32[:].bitcast(I16).rearrange("p (e two) -> p e two", two=2)
            nc.vector.tensor_copy(
                idxbuf[0:16, :].rearrange("p (e one) -> p e one", one=1),
                lo[:, :, 0:1],
            )
            # replicate to the 8 16-partition groups (each dma_gather queue's
            # Q7 core pair streams indices from its own 16-partition group)
            for r in range(1, 8):
                nc.sync.dma_start(idxbuf[16 * r:16 * (r + 1), :], idxbuf[0:16, :])

            for bi in range(nblk):
                s0 = bi * sblk
                wcols = sblk * N // 16  # wrapped columns per block

                # ---- gathers ----------------------------------------
                gbig = gbigp.tile([128, 1, sblk * N], BF16, tag="gbig")
                for c in range(calls):
                    wc0 = bi * wcols + c * (NI // 16)
                    nc.gpsimd.dma_gather(
                        gbig[:, :, c * NI:(c + 1) * NI],
                        tbl[:],
                        idxbuf[:, wc0:wc0 + NI // 16],
                        NI, NI, H,
                        transpose=True,
                        queue_num=c % 4,
                        sbuf_tokens_per_rank=TPR,
                        sbuf_free_dim_per_rank=H * 2,
                    )

                # ---- matmuls: psum[s,k] += g[h, s-slice]^T @ wnt -----
                gv = gbig[:, 0, :]
                for u in range(chunks):
                    ps = psump.tile([128, H], F32, tag="ps")
                    for n in range(N):
                        off = 4096 * u + 512 * (n // 4) + 128 * (n % 4)
                        nc.tensor.matmul(
                            out=ps[:],
                            lhsT=gv[:, off:off + 128],
                            rhs=wnt[:],
                            start=(n == 0),
                            stop=(n == N - 1),
                        )
                    osb = outp.tile([128, H], F32, tag="osb")
                    nc.vector.tensor_copy(osb[:], ps[:])
                    nc.sync.dma_start(
                        out_d[s0 + u * 128:s0 + (u + 1) * 128, :], osb[:]
                    )

    nc.compile()
    return nc


_CACHE: dict[int, object] = {}


def _get_program(S: int):
    if S not in _CACHE:
        _CACHE[S] = build_program(S)
    return _CACHE[S]


def kernel(x, h, g, neighbor_index, neighbor_mask, pos_table, Wn):
    """Full inputs in, full output out. x and g are unused by the math
    (g only provides the zero row shape; x is unused in the reference)."""
    h = np.asarray(h)
    idx = np.asarray(neighbor_index)
    msk = np.asarray(neighbor_mask)
    pos = np.ascontiguousarray(np.asarray(pos_table), dtype=np.float32)
    wn = np.ascontiguousarray(np.asarray(Wn), dtype=np.float32)
    b, s, n = idx.shape
    assert (b, n) == (B, N) and h.shape == (B, s, H)

    nc = _get_program(s)
    in_maps = [
        {
            "h": np.ascontiguousarray(h[c], dtype=np.float32),
            "idx": np.ascontiguousarray(idx[c], dtype=np.int32),
            "msk": np.ascontiguousarray(msk[c], dtype=np.int32),
            "pos": pos,
            "wn": wn,
        }
        for c in range(B)
    ]
    res = run_bass_kernel_spmd(nc, in_maps, core_ids=list(range(B)))
    return np.stack([res.results[c]["out"] for c in range(B)], axis=0)



# revision 4
# speedup vs baseline: 1.2055x; 1.2055x over previous
"""Trainium2 Bass kernel for nn_Neighbor_Mean (gnn message passing).

Math: out[b,s,:] = mean_n( mask[b,s,n] * (T_b[idx[b,s,n]] @ Wn^T) )
 with T_b[v] = pos_table[v] + (h[b][v-1] if v>=1 else 0)   (v in [0, 2049))
Since the mask multiplies matmul outputs and matmul is linear:
 out[b,s,:] = ( (1/N) * sum_n T'_b[idx_eff[b,s,n]] ) @ Wn^T
 where T' has an extra zero row at SINK=2049 and idx_eff = mask ? idx : SINK.

Sharding: data-parallel over batch, one NeuronCore per batch row (B == 8).

Per-core plan (v2 -- descriptor-lean prologue):
 - table T' in SBUF as bf16, packed [128 part, 17*128] (row v at partition
   v%128, free chunk v//128) -- the SBUF-source layout of dma_gather.
   Built with 4 large DMAs (pos rearrange, pos row 2048, h shifted via a
   partition-offset DRAM AP, h boundary rows) + one DVE add.
 - indices/mask loaded CONTIGUOUSLY into [16, S*N/16] (partition = s//128,
   col = (s%128)*N + n): 16 descriptors per tensor instead of 65536 4-byte
   ones. Masked-select against SINK on DVE, int32->int16, replicate to the
   8 16-partition groups by doubling.
 - SBUF->SBUF transposed dma_gather, NI idx/call, 4 SWDGE queues. With the
   contiguous layout, gather position i = 16*col + p covers (s = 128p +
   i//512*?); per 4096-position chunk k the columns are j = 512*w + 16*n + p
   with s = 128p + 8k + w.
 - PE: per chunk, PSUM-accumulate 32 matmuls over n with a strided lhsT AP
   [128h, (w:stride 512, 8), (p:1, 16)]; psum[m, k] with m = 16w + p.
 - copy PSUM->SBUF, DMA out rows with the matching strided DRAM AP.

IMPORTANT: all 2-read DVE ops (copy_predicated, tensor_add) must finish
before any dma_gather runs -- the gather ucode streams its indices through
the POOL/DVE shared SBUF read port, and a concurrent 2-port DVE op corrupts
the stream. All index/table prep happens in the prologue; every gather
transitively depends on it.
"""
import sys

sys.path.insert(0, '/opt/trn_rl_repo')

import numpy as np

import concourse.bacc as bacc
import concourse.bass as bass
import concourse.mybir as mybir
import concourse.tile as tile
from concourse.bass_utils import run_bass_kernel_spmd
from concourse.masks import make_identity

B, N, H = 8, 32, 128
NI = 512             # idxs per dma_gather call
SBLK = 512           # s rows per pipeline block (positions: SBLK*N)
F32 = mybir.dt.float32
I32 = mybir.dt.int32
I16 = mybir.dt.int16
BF16 = mybir.dt.bfloat16


def build_program(S: int = 2048, ni: int = NI):
    VPOS = S + 1                      # pos_table rows; SINK index == VPOS
    NRANKS = (VPOS + 1 + 127) // 128  # table chunks incl. sink row, padded
    nblk = S // SBLK if S >= SBLK else 1
    sblk = min(SBLK, S)
    posn_blk = sblk * N               # gather positions per block
    calls = posn_blk // ni            # gather calls per block
    chunks = posn_blk // 4096         # psum chunks per block (4096 posn each)
    assert S % 128 == 0 and posn_blk % ni == 0 and posn_blk % 4096 == 0

    nc = bacc.Bacc("TRN2", debug=False, num_swdge_queues=4)
    h_d = nc.dram_tensor("h", [S, H], F32, kind="ExternalInput")
    idx_d = nc.dram_tensor("idx", [S, N], I32, kind="ExternalInput")
    msk_d = nc.dram_tensor("msk", [S, N], I32, kind="ExternalInput")
    pos_d = nc.dram_tensor("pos", [VPOS, H], F32, kind="ExternalInput")
    wn_d = nc.dram_tensor("wn", [H, H], F32, kind="ExternalInput")
    out_d = nc.dram_tensor("out", [S, H], F32, kind="ExternalOutput")

    with tile.TileContext(nc) as tc:
        with (
            tc.tile_pool(name="const", bufs=1) as constp,
            tc.tile_pool(name="stage", bufs=1) as stagep,
            tc.tile_pool(name="idxp", bufs=1) as idxp,
            tc.tile_pool(name="gbig", bufs=2) as gbigp,
            tc.tile_pool(name="outp", bufs=4) as outp,
            tc.tile_pool(name="psum", bufs=4, space="PSUM") as psump,
        ):
            # ---- Wn^T * (1/N) in bf16 --------------------------------
            wn_sb = constp.tile([H, H], F32)
            nc.sync.dma_start(wn_sb[:], wn_d[:])
            ident = constp.tile([128, 128], F32)
            make_identity(nc, ident[:])
            wnt_ps = psump.tile([128, H], F32)
            nc.tensor.transpose(out=wnt_ps[:], in_=wn_sb[:], identity=ident[:])
            wnt = constp.tile([H, H], BF16)
            nc.vector.tensor_scalar_mul(wnt[:], wnt_ps[:], 1.0 / N)

            # ---- fused table T' (bf16, gather-packed layout) ---------
            # tbl[p, q*H:(q+1)*H] = T'[q*128 + p, :]
            QF = NRANKS - 1  # full 128-row ranks (last rank is partial)
            pstage = stagep.tile([128, NRANKS * H], F32, tag="pstage")
            hstage = stagep.tile([128, NRANKS * H], F32, tag="hstage")
            # last rank: only (p=0,q=QF) = row S is real; sink row S+1 and
            # pad rows stay zero.
            nc.gpsimd.memset(pstage[:, QF * H:], 0.0)
            nc.gpsimd.memset(hstage[:, QF * H:], 0.0)
            nc.gpsimd.memset(hstage[0:1, 0:H], 0.0)  # v=0: new_h = 0
            # pos rows [0, S) -> pstage[p, q] (v = 128q + p)
            nc.sync.dma_start(
                pstage[:, 0:QF * H].rearrange("p (q e) -> p q e", q=QF),
                pos_d[0:S, :].rearrange("(q p) e -> p q e", p=128),
            )
            # pos row S -> partition 0 of last rank
            nc.scalar.dma_start(pstage[0:1, QF * H:QF * H + H], pos_d[S:S + 1, :])
            # h rows: hstage[p, q] = h[128q + p - 1]
            #  p in [1,128), q in [0,QF): h rows 0..S-2 via partition shift
            nc.scalar.dma_start(
                hstage[1:128, 0:QF * H].rearrange("p (q e) -> p q e", q=QF),
                h_d[0:S, :].rearrange("(q p) e -> p q e", p=128)[0:127, :, :],
            )
            #  p = 0, q in [1,QF]: h rows 128q-1 (incl. v=S -> h[S-1])
            nc.sync.dma_start(
                hstage[0:1, H:(QF + 1) * H].rearrange("p (q e) -> p q e", q=QF),
                h_d[0:S, :].rearrange("(q p) e -> p q e", p=128)[127:128, :, :],
            )
            tbl = constp.tile([128, NRANKS * H], BF16)
            nc.vector.tensor_add(tbl[:], pstage[:], hstage[:])

            # ---- masked indices, contiguous layout -------------------
            # idxw32[p, c] = idx[128*p + c//N, c%N]  (16 partitions)
            acols = S * N // 16
            idxw32 = idxp.tile([16, acols], I32, tag="idxw32")
            mskw32 = idxp.tile([16, acols], I32, tag="mskw32")
            nc.sync.dma_start(
                idxw32[:], idx_d[:].rearrange("(p r) n -> p (r n)", p=16)
            )
            nc.scalar.dma_start(
                mskw32[:], msk_d[:].rearrange("(p r) n -> p (r n)", p=16)
            )
            c_sink = idxp.tile([16, acols], I32, tag="c_sink")
            nc.gpsimd.memset(c_sink[:], VPOS)
            idxe32 = idxp.tile([16, acols], I32, tag="idxe32")
            nc.vector.tensor_copy(idxe32[:], c_sink[:])
            nc.vector.copy_predicated(idxe32[:], mskw32[:], idxw32[:])
            # int32 -> int16 (values < 2^15: take low halves), fused with the
            # (k w n) -> (k n w) permutation so each 4096-position chunk k is
            # ordered n-major: position col = 256k + 8n + w reads
            # idx_eff[s = 128p + 8k + w, n] (source col (8k + w)*N + n).
            idxbuf = idxp.tile([128, acols], I16, tag="idxbuf")
            kchunks = acols // 256
            lo = idxe32[:].bitcast(I16).rearrange(
                "p (k w n two) -> p k n w two", w=8, n=N, two=2
            )
            dst = idxbuf[0:16, :].rearrange(
                "p (k n w one) -> p k n w one", n=N, w=8, one=1
            )
            nc.vector.tensor_copy(dst, lo[:, :, :, :, 0:1])
            # replicate to the 8 16-partition groups by doubling
            nc.sync.dma_start(idxbuf[16:32, :], idxbuf[0:16, :])
            nc.sync.dma_start(idxbuf[32:64, :], idxbuf[0:32, :])
            nc.sync.dma_start(idxbuf[64:128, :], idxbuf[0:64, :])

            for bi in range(nblk):
                # ---- gathers ----------------------------------------
                gbig = gbigp.tile([128, 1, posn_blk], BF16, tag="gbig")
                for c in range(calls):
                    wc0 = (bi * posn_blk + c * ni) // 16
                    nc.gpsimd.dma_gather(
                        gbig[:, :, c * ni:(c + 1) * ni],
                        tbl[:],
                        idxbuf[:, wc0:wc0 + ni // 16],
                        ni, ni, H,
                        transpose=True,
                        queue_num=c % 4,
                        sbuf_tokens_per_rank=128,
                        sbuf_free_dim_per_rank=H * 2,
                    )

                # ---- matmuls: psum[m,k] += g[h, cols]^T @ wnt --------
                # chunk k covers positions [4096k, 4096(k+1)); column
                # j = 128n + 16w + p holds (s = 128p + 8k + w, n).
                gv = gbig[:, 0, :]
                for u in range(chunks):
                    kg = bi * chunks + u  # global chunk id
                    ps = psump.tile([128, H], F32, tag="ps")
                    for n in range(N):
                        off = u * 4096 + 128 * n
                        nc.tensor.matmul(
                            out=ps[:],
                            lhsT=gv[:, off:off + 128],
                            rhs=wnt[:],
                            start=(n == 0),
                            stop=(n == N - 1),
                        )
                    osb = outp.tile([128, H], F32, tag="osb")
                    nc.vector.tensor_copy(osb[:], ps[:])
                    # psum row m = 16w + p -> out row s = 128p + 8k + w
                    dst = out_d[:].rearrange("(p r) e -> p r e", p=16)[
                        :, 8 * kg:8 * kg + 8, :
                    ].rearrange("p w e -> w p e")
                    nc.sync.dma_start(dst, osb[:])

    nc.compile()
    return nc


_CACHE: dict[tuple, object] = {}


def _get_program(S: int, ni: int = NI):
    key = (S, ni)
    if key not in _CACHE:
        _CACHE[key] = build_program(S, ni)
    return _CACHE[key]


def kernel(x, h, g, neighbor_index, neighbor_mask, pos_table, Wn):
    """Full inputs in, full output out. x and g are unused by the math
    (g only provides the zero row shape; x is unused in the reference)."""
    h = np.asarray(h)
    idx = np.asarray(neighbor_index)
    msk = np.asarray(neighbor_mask)
    pos = np.ascontiguousarray(np.asarray(pos_table), dtype=np.float32)
    wn = np.ascontiguousarray(np.asarray(Wn), dtype=np.float32)
    b, s, n = idx.shape
    assert (b, n) == (B, N) and h.shape == (B, s, H)

    nc = _get_program(s)
    in_maps = [
        {
            "h": np.ascontiguousarray(h[c], dtype=np.float32),
            "idx": np.ascontiguousarray(idx[c], dtype=np.int32),
            "msk": np.ascontiguousarray(msk[c], dtype=np.int32),
            "pos": pos,
            "wn": wn,
        }
        for c in range(B)
    ]
    res = run_bass_kernel_spmd(nc, in_maps, core_ids=list(range(B)))
    return np.stack([res.results[c]["out"] for c in range(B)], axis=0)


# revision 13
# speedup vs baseline: 1.3800x; 1.1447x over previous
"""Trainium2 Bass kernel for nn_Neighbor_Mean (gnn message passing).

Math: out[b,s,:] = mean_n( mask[b,s,n] * (T_b[idx[b,s,n]] @ Wn^T) )
 with T_b[v] = pos_table[v] + (h[b][v-1] if v>=1 else 0)   (v in [0, 2049))
Since the mask multiplies matmul outputs and matmul is linear:
 out[b,s,:] = ( (1/N) * sum_n T'_b[idx_eff[b,s,n]] ) @ Wn^T
 where T' has an extra zero row at SINK=2049 and idx_eff = mask ? idx : SINK.

Sharding: data-parallel over batch, one NeuronCore per batch row (B == 8).

Per-core plan (v2 -- descriptor-lean prologue):
 - table T' in SBUF as bf16, packed [128 part, 17*128] (row v at partition
   v%128, free chunk v//128) -- the SBUF-source layout of dma_gather.
   Built with 4 large DMAs (pos rearrange, pos row 2048, h shifted via a
   partition-offset DRAM AP, h boundary rows) + one DVE add.
 - indices/mask loaded CONTIGUOUSLY into [16, S*N/16] (partition = s//128,
   col = (s%128)*N + n): 16 descriptors per tensor instead of 65536 4-byte
   ones. Masked-select against SINK on DVE, int32->int16, replicate to the
   8 16-partition groups by doubling.
 - SBUF->SBUF transposed dma_gather, NI idx/call, 4 SWDGE queues. With the
   contiguous layout, gather position i = 16*col + p covers (s = 128p +
   i//512*?); per 4096-position chunk k the columns are j = 512*w + 16*n + p
   with s = 128p + 8k + w.
 - PE: per chunk, PSUM-accumulate 32 matmuls over n with a strided lhsT AP
   [128h, (w:stride 512, 8), (p:1, 16)]; psum[m, k] with m = 16w + p.
 - copy PSUM->SBUF, DMA out rows with the matching strided DRAM AP.

IMPORTANT: all 2-read DVE ops (copy_predicated, tensor_add) must finish
before any dma_gather runs -- the gather ucode streams its indices through
the POOL/DVE shared SBUF read port, and a concurrent 2-port DVE op corrupts
the stream. All index/table prep happens in the prologue; every gather
transitively depends on it.
"""
import sys

sys.path.insert(0, '/opt/trn_rl_repo')

import numpy as np

import concourse.bacc as bacc
import concourse.bass as bass
import concourse.mybir as mybir
import concourse.tile as tile
from concourse.bass_utils import run_bass_kernel_spmd
from concourse.masks import make_identity

B, N, H = 8, 32, 128
NI = 512             # idxs per dma_gather call
SBLK = 512           # s rows per pipeline block (positions: SBLK*N)
F32 = mybir.dt.float32
I32 = mybir.dt.int32
I16 = mybir.dt.int16
BF16 = mybir.dt.bfloat16


def build_program(S: int = 2048, ni: int = NI):
    VPOS = S + 1                      # pos_table rows; SINK index == VPOS
    NRANKS = (VPOS + 1 + 127) // 128  # table chunks incl. sink row, padded
    nblk = S // SBLK if S >= SBLK else 1
    sblk = min(SBLK, S)
    posn_blk = sblk * N               # gather positions per block
    calls = posn_blk // ni            # gather calls per block
    chunks = posn_blk // 4096         # psum chunks per block (4096 posn each)
    assert S % 128 == 0 and posn_blk % ni == 0 and posn_blk % 4096 == 0

    nc = bacc.Bacc("TRN2", debug=False, num_swdge_queues=4)
    # nh = reference's new_h = concat([zeros(1,H), h]): one zero row then h.
    nh_d = nc.dram_tensor("nh", [S + 1, H], F32, kind="ExternalInput")
    idx_d = nc.dram_tensor("idx", [S, N], I32, kind="ExternalInput")
    msk_d = nc.dram_tensor("msk", [S, N], I32, kind="ExternalInput")
    pos_d = nc.dram_tensor("pos", [VPOS, H], F32, kind="ExternalInput")
    wn_d = nc.dram_tensor("wn", [H, H], F32, kind="ExternalInput")
    out_d = nc.dram_tensor("out", [S, H], F32, kind="ExternalOutput")

    with tile.TileContext(nc) as tc:
        with (
            tc.tile_pool(name="const", bufs=1) as constp,
            tc.tile_pool(name="stage", bufs=1) as stagep,
            tc.tile_pool(name="idxp", bufs=1) as idxp,
            tc.tile_pool(name="gbig", bufs=2) as gbigp,
            tc.tile_pool(name="outp", bufs=4) as outp,
            tc.tile_pool(name="psum", bufs=4, space="PSUM") as psump,
        ):
            # ---- Wn^T * (1/N) in bf16 --------------------------------
            wn_sb = constp.tile([H, H], F32)
            nc.sync.dma_start(wn_sb[:], wn_d[:])
            ident = constp.tile([128, 128], F32)
            make_identity(nc, ident[:])
            wnt_ps = psump.tile([128, H], F32)
            nc.tensor.transpose(out=wnt_ps[:], in_=wn_sb[:], identity=ident[:])
            wnt = constp.tile([H, H], BF16)
            nc.vector.tensor_scalar_mul(wnt[:], wnt_ps[:], 1.0 / N)

            # ---- fused table T' (bf16, gather-packed layout) ---------
            # Table slot g(v) = 128*(v%17) + v//17, i.e. partition p = v//17,
            # rank r = v%17: tbl[p, r*H:(r+1)*H] = T'[17p + r]. Indices are
            # host-remapped to g(idx); sink g(S+1) stays a zero row. Rows
            # land 17-consecutive per partition -> the staging loads are one
            # big contiguous descriptor per partition.
            PFULL = S // NRANKS          # partitions fully covered by rows
            PREST = S - PFULL * NRANKS   # leftover rows on partition PFULL
            pstage = stagep.tile([128, NRANKS * H], F32, tag="pstage")
            hstage = stagep.tile([128, NRANKS * H], F32, tag="hstage")
            # gpsimd memsets need 32-partition (quadrant) alignment; the main
            # loads overwrite partitions [PAL, PFULL) afterwards.
            PAL = (PFULL // 32) * 32
            nc.gpsimd.memset(pstage[PAL:128, :], 0.0)
            nc.gpsimd.memset(hstage[PAL:128, :], 0.0)
            nc.sync.dma_start(
                pstage[0:PFULL, :],
                pos_d[0:PFULL * NRANKS, :].rearrange("(p r) e -> p (r e)", p=PFULL),
            )
            nc.sync.dma_start(
                hstage[0:PFULL, :],
                nh_d[0:PFULL * NRANKS, :].rearrange("(p r) e -> p (r e)", p=PFULL),
            )
            # leftover rows [PFULL*17, S] -> partition PFULL, ranks 0..PREST
            nc.sync.dma_start(
                pstage[PFULL:PFULL + 1, 0:(PREST + 1) * H],
                pos_d[PFULL * NRANKS:S + 1, :].rearrange("(p r) e -> p (r e)", p=1),
            )
            nc.sync.dma_start(
                hstage[PFULL:PFULL + 1, 0:(PREST + 1) * H],
                nh_d[PFULL * NRANKS:S + 1, :].rearrange("(p r) e -> p (r e)", p=1),
            )
            tbl = constp.tile([128, NRANKS * H], BF16)
            nc.vector.tensor_add(tbl[:], pstage[:], hstage[:])

            # ---- masked indices, contiguous layout -------------------
            # idxw32[p, c] = idx[128*p + c//N, c%N]  (16 partitions)
            acols = S * N // 16
            idxw32 = idxp.tile([16, acols], I32, tag="idxw32")
            mskw32 = idxp.tile([16, acols], I32, tag="mskw32")
            nc.sync.dma_start(
                idxw32[:], idx_d[:].rearrange("(p r) n -> p (r n)", p=16)
            )
            nc.sync.dma_start(
                mskw32[:], msk_d[:].rearrange("(p r) n -> p (r n)", p=16)
            )
            c_sink = idxp.tile([16, acols], I32, tag="c_sink")
            sink_g = 128 * (VPOS % NRANKS) + VPOS // NRANKS
            nc.gpsimd.memset(c_sink[:], sink_g)
            idxe32 = idxp.tile([16, acols], I32, tag="idxe32")
            nc.vector.tensor_copy(idxe32[:], c_sink[:])
            nc.vector.copy_predicated(idxe32[:], mskw32[:], idxw32[:])
            # int32 -> int16 (values < 2^15: take low halves), fused with the
            # (k w n) -> (k n w) permutation so each 4096-position chunk k is
            # ordered n-major: position col = 256k + 8n + w reads
            # idx_eff[s = 128p + 8k + w, n] (source col (8k + w)*N + n).
            idxbuf = idxp.tile([128, acols], I16, tag="idxbuf")
            kchunks = acols // 256
            lo = idxe32[:].bitcast(I16).rearrange(
                "p (k w n two) -> p k n w two", w=8, n=N, two=2
            )
            dst = idxbuf[0:16, :].rearrange(
                "p (k n w one) -> p k n w one", n=N, w=8, one=1
            )
            nc.vector.tensor_copy(dst, lo[:, :, :, :, 0:1])
            # replicate to the 8 16-partition groups by doubling
            nc.sync.dma_start(idxbuf[16:32, :], idxbuf[0:16, :])
            nc.sync.dma_start(idxbuf[32:64, :], idxbuf[0:32, :])
            nc.sync.dma_start(idxbuf[64:128, :], idxbuf[0:64, :])

            for bi in range(nblk):
                # ---- gathers ----------------------------------------
                gbig = gbigp.tile([128, 1, posn_blk], BF16, tag="gbig")
                for c in range(calls):
                    wc0 = (bi * posn_blk + c * ni) // 16
                    nc.gpsimd.dma_gather(
                        gbig[:, :, c * ni:(c + 1) * ni],
                        tbl[:],
                        idxbuf[:, wc0:wc0 + ni // 16],
                        ni, ni, H,
                        transpose=True,
                        queue_num=c % 4,
                        sbuf_tokens_per_rank=128,
                        sbuf_free_dim_per_rank=H * 2,
                    )

                # ---- matmuls: psum[m,k] += g[h, cols]^T @ wnt --------
                # chunk k covers positions [4096k, 4096(k+1)); column
                # j = 128n + 16w + p holds (s = 128p + 8k + w, n).
                gv = gbig[:, 0, :]
                for u in range(chunks):
                    kg = bi * chunks + u  # global chunk id
                    ps = psump.tile([128, H], F32, tag="ps")
                    for n in range(N):
                        off = u * 4096 + 128 * n
                        nc.tensor.matmul(
                            out=ps[:],
                            lhsT=gv[:, off:off + 128],
                            rhs=wnt[:],
                            start=(n == 0),
                            stop=(n == N - 1),
                        )
                    osb = outp.tile([128, H], F32, tag="osb")
                    nc.vector.tensor_copy(osb[:], ps[:])
                    # psum row m = 16w + p -> out row s = 128p + 8k + w
                    dst = out_d[:].rearrange("(p r) e -> p r e", p=16)[
                        :, 8 * kg:8 * kg + 8, :
                    ].rearrange("p w e -> w p e")
                    nc.sync.dma_start(dst, osb[:])

    nc.compile()
    return nc


_CACHE: dict[tuple, object] = {}


def _get_program(S: int, ni: int = NI):
    key = (S, ni)
    if key not in _CACHE:
        _CACHE[key] = build_program(S, ni)
    return _CACHE[key]


def kernel(x, h, g, neighbor_index, neighbor_mask, pos_table, Wn):
    """Full inputs in, full output out. x and g are unused by the math
    (g only provides the zero row shape; x is unused in the reference)."""
    h = np.asarray(h)
    idx = np.asarray(neighbor_index)
    msk = np.asarray(neighbor_mask)
    pos = np.ascontiguousarray(np.asarray(pos_table), dtype=np.float32)
    wn = np.ascontiguousarray(np.asarray(Wn), dtype=np.float32)
    b, s, n = idx.shape
    assert (b, n) == (B, N) and h.shape == (B, s, H)

    nc = _get_program(s)
    # nh = reference's new_h = concat([zeros(1,H), h]) per batch row
    nh = np.concatenate(
        [np.zeros((B, 1, H), dtype=np.float32), h.astype(np.float32)], axis=1
    )
    # remap indices to table slots g(v) = 128*(v%17) + v//17 (see table
    # layout in build_program)
    nranks = (s + 2 + 127) // 128
    idx_g = (128 * (idx % nranks) + idx // nranks).astype(np.int32)
    in_maps = [
        {
            "nh": np.ascontiguousarray(nh[c]),
            "idx": np.ascontiguousarray(idx_g[c]),
            "msk": np.ascontiguousarray(msk[c], dtype=np.int32),
            "pos": pos,
            "wn": wn,
        }
        for c in range(B)
    ]
    res = run_bass_kernel_spmd(nc, in_maps, core_ids=list(range(B)))
    return np.stack([res.results[c]["out"] for c in range(B)], axis=0)


# revision 19
# speedup vs baseline: 1.4242x; 1.0320x over previous
"""Trainium2 Bass kernel for nn_Neighbor_Mean (gnn message passing).

Math: out[b,s,:] = mean_n( mask[b,s,n] * (T_b[idx[b,s,n]] @ Wn^T) )
 with T_b[v] = pos_table[v] + (h[b][v-1] if v>=1 else 0)   (v in [0, 2049))
Since the mask multiplies matmul outputs and matmul is linear:
 out[b,s,:] = ( (1/N) * sum_n T'_b[idx_eff[b,s,n]] ) @ Wn^T
 where T' has an extra zero row at SINK=S+1 and idx_eff = mask ? idx : SINK.

Sharding: data-parallel over batch, one NeuronCore per batch row (B == 8).

Per-core plan (v5):
 - table T' in SBUF as bf16, slot g(v) = 128*(v%17) + v//17 (partition
   p = v//17, rank r = v%17) -- both staging loads are one contiguous
   8.5KB descriptor per partition. Indices are host-remapped to g(idx);
   host also passes nh = concat([zeros, h]) so no cross-partition shift.
 - indices/mask loaded contiguously into [16, S*N/16] (partition = s//128,
   col = (s%128)*N + n). Masked-select against the sink slot on DVE,
   int32->int16 fused with a (k w n)->(k n w) permutation so each
   4096-position chunk is n-major, replicate to 8 16-partition groups.
 - a dummy 128-idx dma_gather early in the prologue preloads the Q7
   ext-isa library so the first real gather doesn't pay the reload.
 - SBUF->SBUF transposed dma_gather, 512 idx/call (ucode ring ceiling),
   4 SWDGE queues; desc-gen on GpSimd is the serial bottleneck
   (~2.9us/call, ~373us total).
 - PE: per 4096-position chunk, PSUM-accumulate 32 matmuls (contiguous
   128-column slices); psum row m = 16w + p -> out row s = 128p + 8k + w
   via a strided DRAM AP on the output DMA.

IMPORTANT: all 2-read DVE ops (copy_predicated, tensor_add) must finish
before any real dma_gather runs -- the gather ucode streams its indices
through the POOL/DVE shared SBUF read port, and a concurrent 2-port DVE
op corrupts the stream. All real gathers transitively depend on the DVE
prologue via tbl/idxbuf; the dummy gather's output is never read.
"""
import sys

sys.path.insert(0, '/opt/trn_rl_repo')

import numpy as np

import concourse.bacc as bacc
import concourse.bass as bass
import concourse.mybir as mybir
import concourse.tile as tile
from concourse.bass_utils import run_bass_kernel_spmd
from concourse.masks import make_identity

B, N, H = 8, 32, 128
NI = 512             # idxs per dma_gather call (ucode ring ceiling)
SBLK = 256           # s rows per pipeline block (positions: SBLK*N)
F32 = mybir.dt.float32
I32 = mybir.dt.int32
I16 = mybir.dt.int16
BF16 = mybir.dt.bfloat16


def build_program(S: int = 2048, ni: int = NI):
    VPOS = S + 1                      # pos_table rows; sink index == VPOS
    NRANKS = (VPOS + 1 + 127) // 128  # table ranks (17 for S=2048)
    nblk = S // SBLK if S >= SBLK else 1
    sblk = min(SBLK, S)
    posn_blk = sblk * N               # gather positions per block
    calls = posn_blk // ni            # gather calls per block
    chunks = posn_blk // 4096         # psum chunks per block (4096 posn each)
    assert S % 128 == 0 and posn_blk % ni == 0 and posn_blk % 4096 == 0
    assert ni <= 512, "dma_gather ucode ring ceiling is 512 idxs/call"

    nc = bacc.Bacc("TRN2", debug=False, num_swdge_queues=4)
    # nh = reference's new_h = concat([zeros(1,H), h]): one zero row then h.
    nh_d = nc.dram_tensor("nh", [S + 1, H], F32, kind="ExternalInput")
    idx_d = nc.dram_tensor("idx", [S, N], I32, kind="ExternalInput")
    msk_d = nc.dram_tensor("msk", [S, N], I32, kind="ExternalInput")
    pos_d = nc.dram_tensor("pos", [VPOS, H], F32, kind="ExternalInput")
    wn_d = nc.dram_tensor("wn", [H, H], F32, kind="ExternalInput")
    out_d = nc.dram_tensor("out", [S, H], F32, kind="ExternalOutput")

    with tile.TileContext(nc) as tc:
        with (
            tc.tile_pool(name="const", bufs=1) as constp,
            tc.tile_pool(name="stage", bufs=1) as stagep,
            tc.tile_pool(name="idxp", bufs=1) as idxp,
            tc.tile_pool(name="gbig", bufs=2) as gbigp,
            tc.tile_pool(name="outp", bufs=4) as outp,
            tc.tile_pool(name="psum", bufs=4, space="PSUM") as psump,
        ):
            acols = S * N // 16
            # ---- index/mask loads first (head of the sync ring) ------
            idxw32 = idxp.tile([16, acols], I32, tag="idxw32")
            mskw32 = idxp.tile([16, acols], I32, tag="mskw32")
            nc.sync.dma_start(
                idxw32[:], idx_d[:].rearrange("(p r) n -> p (r n)", p=16)
            )
            nc.sync.dma_start(
                mskw32[:], msk_d[:].rearrange("(p r) n -> p (r n)", p=16)
            )

            # ---- gpsimd: sink fill, then dummy gather (ucode preload) -
            idxe32 = idxp.tile([16, acols], I32, tag="idxe32")
            sink_g = 128 * (VPOS % NRANKS) + VPOS // NRANKS
            nc.gpsimd.memset(idxe32[:], sink_g)
            dummy_src = constp.tile([128, 128], BF16, tag="dsrc")
            dummy_idx = constp.tile([128, 8], I16, tag="didx")
            dummy_out = constp.tile([128, 1, 128], BF16, tag="dout")
            nc.gpsimd.memset(dummy_idx[:], 0)
            nc.gpsimd.memset(dummy_src[:], 0.0)
            nc.gpsimd.dma_gather(
                dummy_out[:], dummy_src[:], dummy_idx[:], 128, 128, H,
                transpose=True, queue_num=0,
                sbuf_tokens_per_rank=128, sbuf_free_dim_per_rank=H * 2,
            )
            ident = constp.tile([128, 128], F32)
            make_identity(nc, ident[:])

            # ---- fused table T' (bf16, gather-packed layout) ---------
            # slot g(v) = 128*(v%17) + v//17: tbl[p, r*H:(r+1)*H] =
            # T'[17p + r]; one contiguous descriptor per partition.
            PFULL = S // NRANKS          # partitions fully covered by rows
            PREST = S - PFULL * NRANKS   # leftover rows on partition PFULL
            pstage = stagep.tile([128, NRANKS * H], F32, tag="pstage")
            hstage = stagep.tile([128, NRANKS * H], F32, tag="hstage")
            nc.scalar.dma_start(
                pstage[0:PFULL, :],
                pos_d[0:PFULL * NRANKS, :].rearrange("(p r) e -> p (r e)", p=PFULL),
            )
            nc.scalar.dma_start(
                hstage[0:PFULL, :],
                nh_d[0:PFULL * NRANKS, :].rearrange("(p r) e -> p (r e)", p=PFULL),
            )
            # leftover rows [PFULL*17, S] -> partition PFULL, ranks 0..PREST
            nc.scalar.dma_start(
                pstage[PFULL:PFULL + 1, 0:(PREST + 1) * H],
                pos_d[PFULL * NRANKS:S + 1, :].rearrange("(p r) e -> p (r e)", p=1),
            )
            nc.scalar.dma_start(
                hstage[PFULL:PFULL + 1, 0:(PREST + 1) * H],
                nh_d[PFULL * NRANKS:S + 1, :].rearrange("(p r) e -> p (r e)", p=1),
            )
            # sink slot (partition VPOS//17, rank VPOS%17) must be zero in
            # tbl; compute memsets can't start at partition 120, so DMA the
            # zero row nh[0] into both stagings instead.
            sp, sr = VPOS // NRANKS, VPOS % NRANKS
            nc.scalar.dma_start(pstage[sp:sp + 1, sr * H:(sr + 1) * H], nh_d[0:1, :])
            nc.scalar.dma_start(hstage[sp:sp + 1, sr * H:(sr + 1) * H], nh_d[0:1, :])
            wn_sb = constp.tile([H, H], F32)
            nc.scalar.dma_start(wn_sb[:], wn_d[:])

            # ---- DVE chain: select -> convert -> (table add) ---------
            nc.vector.copy_predicated(idxe32[:], mskw32[:], idxw32[:])
            # int32 -> int16 (values < 2^15: take low halves), fused with
            # the (k w n) -> (k n w) permutation: position col = 256k +
            # 8n + w reads idx_eff[s = 128p + 8k + w, n].
            idxbuf = idxp.tile([128, acols], I16, tag="idxbuf")
            lo = idxe32[:].bitcast(I16).rearrange(
                "p (k w n two) -> p k n w two", w=8, n=N, two=2
            )
            dst = idxbuf[0:16, :].rearrange(
                "p (k n w one) -> p k n w one", n=N, w=8, one=1
            )
            nc.vector.tensor_copy(dst, lo[:, :, :, :, 0:1])
            # replicate to the 8 16-partition groups by doubling
            nc.sync.dma_start(idxbuf[16:32, :], idxbuf[0:16, :])
            nc.sync.dma_start(idxbuf[32:64, :], idxbuf[0:32, :])
            nc.sync.dma_start(idxbuf[64:128, :], idxbuf[0:64, :])

            tbl = constp.tile([128, NRANKS * H], BF16)
            nc.vector.tensor_add(tbl[:], pstage[:], hstage[:])

            # ---- Wn^T * (1/N) in bf16 --------------------------------
            wnt_ps = psump.tile([128, H], F32)
            nc.tensor.transpose(out=wnt_ps[:], in_=wn_sb[:], identity=ident[:])
            wnt = constp.tile([H, H], BF16)
            nc.vector.tensor_scalar_mul(wnt[:], wnt_ps[:], 1.0 / N)

            for bi in range(nblk):
                # ---- gathers ----------------------------------------
                gbig = gbigp.tile([128, 1, posn_blk], BF16, tag="gbig")
                for c in range(calls):
                    wc0 = (bi * posn_blk + c * ni) // 16
                    nc.gpsimd.dma_gather(
                        gbig[:, :, c * ni:(c + 1) * ni],
                        tbl[:],
                        idxbuf[:, wc0:wc0 + ni // 16],
                        ni, ni, H,
                        transpose=True,
                        queue_num=c % 4,
                        sbuf_tokens_per_rank=128,
                        sbuf_free_dim_per_rank=H * 2,
                    )

                # ---- matmuls: psum[m,k] += g[h, cols]^T @ wnt --------
                # chunk k covers positions [4096k, 4096(k+1)); column
                # j = 128n + 16w + p holds (s = 128p + 8k + w, n).
                gv = gbig[:, 0, :]
                for u in range(chunks):
                    kg = bi * chunks + u  # global chunk id
                    ps = psump.tile([128, H], F32, tag="ps")
                    for n in range(N):
                        off = u * 4096 + 128 * n
                        nc.tensor.matmul(
                            out=ps[:],
                            lhsT=gv[:, off:off + 128],
                            rhs=wnt[:],
                            start=(n == 0),
                            stop=(n == N - 1),
                        )
                    osb = outp.tile([128, H], F32, tag="osb")
                    nc.vector.tensor_copy(osb[:], ps[:])
                    # psum row m = 16w + p -> out row s = 128p + 8k + w
                    dst2 = out_d[:].rearrange("(p r) e -> p r e", p=16)[
                        :, 8 * kg:8 * kg + 8, :
                    ].rearrange("p w e -> w p e")
                    nc.sync.dma_start(dst2, osb[:])

    nc.compile()
    return nc


_CACHE: dict[tuple, object] = {}


def _get_program(S: int, ni: int = NI):
    key = (S, ni)
    if key not in _CACHE:
        _CACHE[key] = build_program(S, ni)
    return _CACHE[key]


def kernel(x, h, g, neighbor_index, neighbor_mask, pos_table, Wn):
    """Full inputs in, full output out. x and g are unused by the math
    (g only provides the zero row shape; x is unused in the reference)."""
    h = np.asarray(h)
    idx = np.asarray(neighbor_index)
    msk = np.asarray(neighbor_mask)
    pos = np.ascontiguousarray(np.asarray(pos_table), dtype=np.float32)
    wn = np.ascontiguousarray(np.asarray(Wn), dtype=np.float32)
    b, s, n = idx.shape
    assert (b, n) == (B, N) and h.shape == (B, s, H)

    nc = _get_program(s)
    # nh = reference's new_h = concat([zeros(1,H), h]) per batch row
    nh = np.concatenate(
        [np.zeros((B, 1, H), dtype=np.float32), h.astype(np.float32)], axis=1
    )
    # remap indices to table slots g(v) = 128*(v%17) + v//17 (see table
    # layout in build_program)
    nranks = (s + 2 + 127) // 128
    idx_g = (128 * (idx % nranks) + idx // nranks).astype(np.int32)
    in_maps = [
        {
            "nh": np.ascontiguousarray(nh[c]),
            "idx": np.ascontiguousarray(idx_g[c]),
            "msk": np.ascontiguousarray(msk[c], dtype=np.int32),
            "pos": pos,
            "wn": wn,
        }
        for c in range(B)
    ]
    res = run_bass_kernel_spmd(nc, in_maps, core_ids=list(range(B)))
    return np.stack([res.results[c]["out"] for c in range(B)], axis=0)


# revision 20
# speedup vs baseline: 1.4732x; 1.0344x over previous
"""Trainium2 Bass kernel for nn_Neighbor_Mean (gnn message passing).

Math: out[b,s,:] = mean_n( mask[b,s,n] * (T_b[idx[b,s,n]] @ Wn^T) )
 with T_b[v] = pos_table[v] + (h[b][v-1] if v>=1 else 0)   (v in [0, 2049))
Since the mask multiplies matmul outputs and matmul is linear:
 out[b,s,:] = ( (1/N) * sum_n T'_b[idx_eff[b,s,n]] ) @ Wn^T
 where T' has an extra zero row at SINK=S+1 and idx_eff = mask ? idx : SINK.

Sharding: data-parallel over batch, one NeuronCore per batch row (B == 8).

Per-core plan (v6):
 - table T' in SBUF as bf16, slot g(v) = 128*(v%17) + v//17 (partition
   p = v//17, rank r = v%17). The host pads pos/new_h with zero rows to
   17*128 rows, so each staging load is ONE contiguous-per-partition DMA
   (128 descriptors of 8.5KB); host remaps indices to g(idx) and passes
   nh = concat([zeros, h]). The sink slot is zero via the padding.
 - indices/mask loaded contiguously into [16, S*N/16] int32 (partition =
   s//128, col = (s%128)*N + n). ONE DVE copy_predicated on int16 views
   does masked-select + int32->int16 + the (k w n)->(k n w) permutation
   in a single pass into a sink-prefilled idxbuf; replicate to the 8
   16-partition groups by doubling.
 - gpsimd.load_library(mlp) issued first so the Q7 ext-isa reload
   (~12us) overlaps the prologue instead of stalling the first gather.
 - SBUF->SBUF transposed dma_gather, 512 idx/call (ucode ring ceiling),
   4 SWDGE queues; desc-gen on GpSimd is the serial bottleneck
   (~2.9us/call, ~370us total).
 - pipeline at 4096-position chunk granularity (8 calls + 32 matmuls per
   chunk, 4 chunk tiles in flight): psum row m = 16w + p -> out row
   s = 128p + 8k + w via a strided DRAM AP on the output DMA.

IMPORTANT: all 2-read DVE ops (copy_predicated, tensor_add) must finish
before any dma_gather runs -- the gather ucode streams its indices
through the POOL/DVE shared SBUF read port, and a concurrent 2-port DVE
op corrupts the stream. All gathers transitively depend on the DVE
prologue via tbl/idxbuf.
"""
import sys

sys.path.insert(0, '/opt/trn_rl_repo')

import numpy as np

import concourse.bacc as bacc
import concourse.bass as bass
import concourse.mybir as mybir
import concourse.tile as tile
from concourse import library_config
from concourse.bass_utils import run_bass_kernel_spmd
from concourse.masks import make_identity

B, N, H = 8, 32, 128
NI = 512             # idxs per dma_gather call (ucode ring ceiling)
CHUNK = 4096         # gather positions per pipeline chunk (= 128 s rows)
F32 = mybir.dt.float32
I32 = mybir.dt.int32
I16 = mybir.dt.int16
BF16 = mybir.dt.bfloat16


def build_program(S: int = 2048, ni: int = NI):
    VPOS = S + 1                      # pos_table rows; sink index == VPOS
    NRANKS = (VPOS + 1 + 127) // 128  # table ranks (17 for S=2048)
    nchunk = S * N // CHUNK
    calls = CHUNK // ni               # gather calls per chunk
    assert S % 128 == 0 and CHUNK % ni == 0
    assert ni <= 512, "dma_gather ucode ring ceiling is 512 idxs/call"

    nc = bacc.Bacc("TRN2", debug=False, num_swdge_queues=4)
    # nh = reference's new_h = concat([zeros(1,H), h]) padded with zero rows
    # to NRANKS*128 rows; pos likewise zero-padded.
    nh_d = nc.dram_tensor("nh", [NRANKS * 128, H], F32, kind="ExternalInput")
    idx_d = nc.dram_tensor("idx", [S, N], I32, kind="ExternalInput")
    msk_d = nc.dram_tensor("msk", [S, N], I32, kind="ExternalInput")
    pos_d = nc.dram_tensor("pos", [NRANKS * 128, H], F32, kind="ExternalInput")
    wn_d = nc.dram_tensor("wn", [H, H], F32, kind="ExternalInput")
    out_d = nc.dram_tensor("out", [S, H], F32, kind="ExternalOutput")

    with tile.TileContext(nc) as tc:
        with (
            tc.tile_pool(name="const", bufs=1) as constp,
            tc.tile_pool(name="stage", bufs=1) as stagep,
            tc.tile_pool(name="idxp", bufs=1) as idxp,
            tc.tile_pool(name="gbig", bufs=4) as gbigp,
            tc.tile_pool(name="outp", bufs=4) as outp,
            tc.tile_pool(name="psum", bufs=4, space="PSUM") as psump,
        ):
            # preload the Q7 ext-isa library holding DMAGatherAnt so the
            # ~12us IRAM reload overlaps the rest of the prologue
            nc.gpsimd.load_library(library_config.mlp)

            acols = S * N // 16
            # ---- index/mask loads (head of the sync ring) ------------
            idxw32 = idxp.tile([16, acols], I32, tag="idxw32")
            mskw32 = idxp.tile([16, acols], I32, tag="mskw32")
            nc.sync.dma_start(
                idxw32[:], idx_d[:].rearrange("(p r) n -> p (r n)", p=16)
            )
            nc.sync.dma_start(
                mskw32[:], msk_d[:].rearrange("(p r) n -> p (r n)", p=16)
            )

            # ---- table staging loads ---------------------------------
            # slot g(v) = 128*(v%17) + v//17: tbl[p, r*H:(r+1)*H] =
            # T'[17p + r]; one contiguous descriptor per partition.
            pstage = stagep.tile([128, NRANKS * H], F32, tag="pstage")
            hstage = stagep.tile([128, NRANKS * H], F32, tag="hstage")
            nc.sync.dma_start(
                pstage[:], pos_d[:].rearrange("(p r) e -> p (r e)", p=128)
            )
            nc.scalar.dma_start(
                hstage[:], nh_d[:].rearrange("(p r) e -> p (r e)", p=128)
            )
            wn_sb = constp.tile([H, H], F32)
            nc.scalar.dma_start(wn_sb[:], wn_d[:])

            # ---- fused masked-select + int16 convert + permutation ---
            # idxbuf[0:16] prefilled with the sink slot; ONE
            # copy_predicated on int16 views then writes g(idx) where
            # mask!=0, applying the (k w n) -> (k n w) permutation:
            # position col = 256k + 8n + w reads idx[s = 128p + 8k + w, n].
            idxbuf = idxp.tile([128, acols], I16, tag="idxbuf")
            sink_g = 128 * (VPOS % NRANKS) + VPOS // NRANKS
            nc.vector.memset(idxbuf[0:16, :], sink_g)
            dst = idxbuf[0:16, :].rearrange(
                "p (k n w one) -> p k n w one", n=N, w=8, one=1
            )
            data = idxw32[:].bitcast(I16).rearrange(
                "p (k w n two) -> p k n w two", w=8, n=N, two=2
            )
            mask = mskw32[:].bitcast(I16).rearrange(
                "p (k w n two) -> p k n w two", w=8, n=N, two=2
            )
            nc.vector.copy_predicated(
                dst, mask[:, :, :, :, 0:1], data[:, :, :, :, 0:1]
            )
            # replicate to the 8 16-partition groups by doubling
            nc.sync.dma_start(idxbuf[16:32, :], idxbuf[0:16, :])
            nc.sync.dma_start(idxbuf[32:64, :], idxbuf[0:32, :])
            nc.sync.dma_start(idxbuf[64:128, :], idxbuf[0:64, :])

            # ---- table add (DVE) + Wn^T * (1/N) ----------------------
            tbl = constp.tile([128, NRANKS * H], BF16)
            nc.vector.tensor_add(tbl[:], pstage[:], hstage[:])
            ident = constp.tile([128, 128], F32)
            make_identity(nc, ident[:])
            wnt_ps = psump.tile([128, H], F32)
            nc.tensor.transpose(out=wnt_ps[:], in_=wn_sb[:], identity=ident[:])
            wnt = constp.tile([H, H], BF16)
            nc.scalar.mul(wnt[:], wnt_ps[:], 1.0 / N)

            for kg in range(nchunk):
                # ---- gathers: chunk kg covers positions [4096kg, ...) -
                gb = gbigp.tile([128, 1, CHUNK], BF16, tag="gb")
                for c in range(calls):
                    ci = kg * calls + c
                    nc.gpsimd.dma_gather(
                        gb[:, :, c * ni:(c + 1) * ni],
                        tbl[:],
                        idxbuf[:, ci * (ni // 16):(ci + 1) * (ni // 16)],
                        ni, ni, H,
                        transpose=True,
                        queue_num=ci % 4,
                        sbuf_tokens_per_rank=128,
                        sbuf_free_dim_per_rank=H * 2,
                    )

                # ---- matmuls: psum[m,k] += g[h, cols]^T @ wnt --------
                # column j = 128n + 16w + p holds (s = 128p + 8kg + w, n)
                gv = gb[:, 0, :]
                ps = psump.tile([128, H], F32, tag="ps")
                for n in range(N):
                    nc.tensor.matmul(
                        out=ps[:],
                        lhsT=gv[:, 128 * n:128 * n + 128],
                        rhs=wnt[:],
                        start=(n == 0),
                        stop=(n == N - 1),
                    )
                osb = outp.tile([128, H], F32, tag="osb")
                nc.vector.tensor_copy(osb[:], ps[:])
                # psum row m = 16w + p -> out row s = 128p + 8kg + w
                dst2 = out_d[:].rearrange("(p r) e -> p r e", p=16)[
                    :, 8 * kg:8 * kg + 8, :
                ].rearrange("p w e -> w p e")
                nc.sync.dma_start(dst2, osb[:])

    nc.compile()
    return nc


_CACHE: dict[tuple, object] = {}


def _get_program(S: int, ni: int = NI):
    key = (S, ni)
    if key not in _CACHE:
        _CACHE[key] = build_program(S, ni)
    return _CACHE[key]


def _prep_host(h, idx, pos, s):
    """Host-side layout prep: padded new_h/pos tables and slot-remapped
    indices (see build_program docstring)."""
    nranks = (s + 2 + 127) // 128
    rows = nranks * 128
    nh = np.zeros((B, rows, H), dtype=np.float32)
    nh[:, 1:s + 1] = h
    pos_pad = np.zeros((rows, H), dtype=np.float32)
    pos_pad[:s + 1] = pos
    idx_g = (128 * (idx % nranks) + idx // nranks).astype(np.int32)
    return nh, pos_pad, idx_g


def kernel(x, h, g, neighbor_index, neighbor_mask, pos_table, Wn):
    """Full inputs in, full output out. x and g are unused by the math
    (g only provides the zero row shape; x is unused in the reference)."""
    h = np.asarray(h, dtype=np.float32)
    idx = np.asarray(neighbor_index)
    msk = np.asarray(neighbor_mask)
    pos = np.asarray(pos_table, dtype=np.float32)
    wn = np.ascontiguousarray(np.asarray(Wn), dtype=np.float32)
    b, s, n = idx.shape
    assert (b, n) == (B, N) and h.shape == (B, s, H)

    nc = _get_program(s)
    nh, pos_pad, idx_g = _prep_host(h, idx, pos, s)
    in_maps = [
        {
            "nh": np.ascontiguousarray(nh[c]),
            "idx": np.ascontiguousarray(idx_g[c]),
            "msk": np.ascontiguousarray(msk[c], dtype=np.int32),
            "pos": pos_pad,
            "wn": wn,
        }
        for c in range(B)
    ]
    res = run_bass_kernel_spmd(nc, in_maps, core_ids=list(range(B)))
    return np.stack([res.results[c]["out"] for c in range(B)], axis=0)
